# revision 3
# baseline (speedup 1.0000x reference)
"""Binarized linear layer (BLinear) Trainium2 kernel.

Computes y = sign(x) @ sign(W).T + b for x [8192, 2048] f32, W [2048, 2048]
f32, b [2048] f32. Data-parallel across 8 NeuronCores: 1024 tokens per core,
W replicated.

Math notes:
 - sign() in {-1, 0, +1} is exact in fp8e4; TensorE accumulates fp32 in
   PSUM; sums of +-1 over K=2048 are exact even integers |v| <= 2048.
 - Host binarizes and packs both operands to fp8 (1 byte) in contraction-
   major layouts, so the device does no transpose and no sign: per-core HBM
   traffic is 6MB in + 4MB out instead of the 20MB of a bf16-staged
   transpose-on-device pipeline.
 - y is evicted as bf16: even integers <= 512 are exact, (512, 2048] round
   with rel err <= 2^-8 — far inside the 2e-2 gate. Host casts to f32 and
   adds b (zeros here, but kept general).

Device pipeline per core (measured ~58.8us/iter steady-state, PE floor
~53us):
 - fp8 DoubleRow matmuls: stationary = x tile [128ki, 2, 128t] (LDWEIGHTS
   reused across 4 o-banks), moving = W [128ki, 2, 512o], K=256/matmul,
   256 matmuls of FD=512 -> the fp8 DoubleRow streaming roofline.
 - Loads: W chunk kp=0 races on the ACT HWDGE ring while the two phase-L x
   tiles load on the SP ring; W chunks 1-7 follow on SP (512KB each, cadence
   1.43us < 1.66us of PE work unlocked per chunk), then per-tile x slabs for
   phase S. Everything >=256KB contiguous — small/strided DMAs measured
   25% slower end-to-end.
 - Phase L: token tiles 0-1 kp-outer (8 psum banks), chasing the chunk
   stream; phase S: tiles 2-7 kp-inner back-to-back.
 - Evictions alternate ScalarE/VectorE per o-bank (psum banks free within
   ~0.9us of their stop, so the next tile's matmuls never wait), stores
   issue per-bank on the ACT ring.
 - A few dummy matmuls on scratch data at body start/end keep the PE's HAM
   activity window busy across idle gaps (DMA prologue, eviction tail, loop
   barrier) so real matmuls run at 2.4 GHz, not the 1.2 GHz cold clock
   (measured +3.6us/iter without them).
"""

import numpy as np

N_CORES = 8
TOKENS = 8192
D_IN = 2048
D_OUT = 2048
T_CORE = TOKENS // N_CORES  # 1024

P = 128
KO = D_IN // P          # 16 contraction chunks of 128
KP = KO // 2            # 8 DoubleRow steps of K=256
T_TILES = T_CORE // P   # 8 token tiles per core
NB = 512                # matmul free dim / PSUM bank
O_BANKS = D_OUT // NB   # 4
L_TT = 2                # token tiles in phase L (2 x 4 = 8 psum banks)

WARM_HEAD = 4           # dummy MMs at body start (fill DMA prologue)
WARM_TAIL = 8           # dummy MMs at body end (bridge tail+barrier)

_CACHE = {}
LAST_RESULT = None


def _build_bass(loop_n=1, phase="all", warm=True):
    import concourse.mybir as mybir
    import concourse.tile as tile
    from concourse import bacc
    from concourse.bass import ts

    nc = bacc.Bacc(
        "TRN2",
        target_bir_lowering=False,
        debug=False,
        enable_asserts=False,
    )

    f32 = mybir.dt.float32
    bf16 = mybir.dt.bfloat16
    fp8 = mybir.dt.float8e4

    x_d = nc.dram_tensor("x", [P, T_TILES, KO, P], fp8, kind="ExternalInput")
    w_d = nc.dram_tensor("W", [P, KO, D_OUT], fp8, kind="ExternalInput")
    y_d = nc.dram_tensor("y", [T_CORE, D_OUT], bf16, kind="ExternalOutput")

    x_ap = x_d.ap()
    w_ap = w_d.ap()
    y_ap = y_d.ap()

    DR = mybir.MatmulPerfMode.DoubleRow

    with tile.TileContext(nc) as tc:
        with (
            tc.tile_pool(name="persist", bufs=1) as persist,
            tc.tile_pool(name="outp", bufs=4) as out_pool,
            tc.tile_pool(name="psum", bufs=8, space="PSUM") as psum_pool,
        ):
            xb = persist.tile([P, T_TILES, KO, P], fp8, name="xb")
            wb = persist.tile([P, KO, D_OUT], fp8, name="wb")
            scr = persist.tile([P, 2, NB], fp8, name="scr")
            nc.gpsimd.memset(scr[:], 1.0)
            if phase == "mm":
                nc.gpsimd.memset(xb[:], 1.0)
                nc.gpsimd.memset(wb[:], 1.0)

            def warm_mms(n):
                if not warm or n == 0:
                    return
                ps = psum_pool.tile([P, NB], f32, tag="psum", name="psum")
                for _ in range(n):
                    nc.tensor.matmul(
                        ps[:],
                        lhsT=scr[:, :, :P],
                        rhs=scr[:],
                        perf_mode=DR,
                        start=True,
                        stop=True,
                    )

            def mm_group(psums, tt, kp):
                for ob in range(O_BANKS):
                    nc.tensor.matmul(
                        psums[ob][:],
                        lhsT=xb[:, tt, 2 * kp : 2 * kp + 2, :],
                        rhs=wb[:, 2 * kp : 2 * kp + 2, ts(ob, NB)],
                        perf_mode=DR,
                        start=(kp == 0),
                        stop=(kp == KP - 1),
                    )

            def evict(psums, tt):
                o_sb = out_pool.tile([P, D_OUT], bf16, name="o_sb")
                for ob in range(O_BANKS):
                    if ob % 2 == 0:
                        nc.scalar.copy(o_sb[:, ts(ob, NB)], psums[ob][:])
                    else:
                        nc.vector.tensor_copy(o_sb[:, ts(ob, NB)], psums[ob][:])
                    nc.scalar.dma_start(
                        y_ap[ts(tt, P), ts(ob, NB)], o_sb[:, ts(ob, NB)]
                    )

            def body():
                warm_mms(WARM_HEAD)
                if phase != "mm":
                    # W chunk 0 in two o-halves on the ACT ring: the first
                    # matmuls (ob 0-1) unblock after 256KB instead of 512KB
                    nc.scalar.dma_start(
                        wb[:, 0:2, : 2 * NB], w_ap[:, 0:2, : 2 * NB]
                    )
                    nc.scalar.dma_start(
                        wb[:, 0:2, 2 * NB :], w_ap[:, 0:2, 2 * NB :]
                    )
                    for tt in range(L_TT):
                        nc.sync.dma_start(xb[:, tt, :, :], x_ap[:, tt, :, :])
                    for kp in range(1, KP):
                        nc.sync.dma_start(
                            wb[:, 2 * kp : 2 * kp + 2, :],
                            w_ap[:, 2 * kp : 2 * kp + 2, :],
                        )
                    for tt in range(L_TT, T_TILES):
                        nc.sync.dma_start(xb[:, tt, :, :], x_ap[:, tt, :, :])
                if phase == "prep":
                    return

                psums_l = [
                    [
                        psum_pool.tile([P, NB], f32, tag="psum", name="psum")
                        for _ in range(O_BANKS)
                    ]
                    for _ in range(L_TT)
                ]
                for kp in range(KP):
                    for tt in range(L_TT):
                        mm_group(psums_l[tt], tt, kp)
                for tt in range(L_TT):
                    evict(psums_l[tt], tt)

                for tt in range(L_TT, T_TILES):
                    psums = [
                        psum_pool.tile([P, NB], f32, tag="psum", name="psum")
                        for _ in range(O_BANKS)
                    ]
                    for kp in range(KP):
                        mm_group(psums, tt, kp)
                    evict(psums, tt)

                warm_mms(WARM_TAIL)

            if loop_n > 1:
                with tc.For_i(
                    0,
                    loop_n,
                    1,
                    hint_engines=(mybir.EngineType.PE,),
                    staggered_reset=True,
                ):
                    body()
            else:
                body()

    nc.compile()
    return nc


def _get_nc():
    if "nc" not in _CACHE:
        _CACHE["nc"] = _build_bass()
    return _CACHE["nc"]


def host_pack(x, W):
    """-> xh [128ki, 64tt, 16ko, 128tp] (token tiles split per core later),
    wh [128ki, 16ko, 2048o], both fp8 (+-1/0 exact)."""
    import ml_dtypes

    sx = np.sign(x.astype(np.float32))
    sw = np.sign(W.astype(np.float32))
    v = sx.reshape(TOKENS // P, P, KO, P)              # [tt, tp, ko, ki]
    xh = np.ascontiguousarray(
        v.transpose(3, 0, 2, 1).astype(ml_dtypes.float8_e4m3)
    )                                                   # [ki, tt, ko, tp]
    w = sw.T.reshape(KO, P, D_OUT).transpose(1, 0, 2)   # [ki, ko, o]
    wh = np.ascontiguousarray(w.astype(ml_dtypes.float8_e4m3))
    return xh, wh


def kernel(**inputs):
    global LAST_RESULT
    from concourse.bass_utils import run_bass_kernel_spmd

    x = np.asarray(inputs["x"], dtype=np.float32)
    W = np.asarray(inputs["W"], dtype=np.float32)
    b = np.asarray(inputs["b"], dtype=np.float32)

    xh, wh = host_pack(x, W)

    nc = _get_nc()
    in_maps = [
        {
            "x": np.ascontiguousarray(xh[:, c * T_TILES : (c + 1) * T_TILES]),
            "W": wh,
        }
        for c in range(N_CORES)
    ]
    res = run_bass_kernel_spmd(nc, in_maps, core_ids=list(range(N_CORES)))
    LAST_RESULT = res
    y = np.concatenate([r["y"].astype(np.float32) for r in res.results], axis=0)
    return y + b[None, :]
